# revision 30
# baseline (speedup 1.0000x reference)
"""Bass/Trainium2 kernel for nn_BuildLstmUnrollNet (fp8 DoubleRow version).

Problem: 2-layer LSTM, unrolled T=11 steps with per-step (non-shared)
weights, B=8192, R=425, IN=20.  Output block t is the last-layer h
*before* step t, so only steps 0..9 are computed.

Strategy (data-parallel over batch, 8 cores x 1024 rows):
  - Gates are computed batch-major in PSUM with the *transposed state*
    stationary and the weights moving, using fp8(e4m3) DoubleRow
    matmuls (0.5 cycles/output column, K=256 per instruction).
  - Precision recovery: the fp16 state H is kept as an fp8 hi/lo pair
    (Hh = fp8(H), Hl = fp8(H - Hh)); every k-pair runs two DoubleRow
    passes (Hh x Wh, Hl x Wh), giving bf16-class state precision at
    half the bf16 PE cost.  For steps 0..1 (large initial states) a
    third pass (Hh x Wl) also corrects the weight quantization.
  - Packed state rows (K dim), 1024 = 8 chunks of 128:
      [h0 425 | 1 | x 20 | z 66 | h1 425 | z 87]
    Layer 0 contracts chunks 0..3 (2 DR pairs), layer 1 chunks 0..7
    (4 pairs; h1 pairs first so the freshly-computed h0' pipeline can
    finish while h1 contributions accumulate).
  - Weights are host-prepacked: W_eff rows match the state layout,
    scaled x64, quantized e4m3 (hi) with e4m3 residual (lo, steps<2).
  - Cell math: ACT sigmoid(1275)/tanh(425) with input scale 1/64 out
    of PSUM into fp16; DVE 4x tensor_tensor fp16 (2x perf mode);
    c-state fp16.
  - Recurrence transpose: h (fp16, batch-major) bounces through DRAM,
    DMA xbar-transposes back in two 512-row groups, then per-chunk
    converts produce Hh (DVE copy fp16->fp8) and Hl (GPSIMD
    tensor_tensor subtract -> fp8).
  - Output: h1 fp16 written per step; host converts to fp32 (block 0
    comes from the initial state on the host, exact).

kernel(**inputs) takes full-size numpy inputs, packs/shards on the
host, runs the program SPMD on cores 0..7 and reassembles the full
[8192, 4675] fp32 output.
"""

import numpy as np
import ml_dtypes

F8 = ml_dtypes.float8_e4m3
F16 = np.float16

B = 8192
NCORES = 8
BC = B // NCORES          # batch rows per core (1024)
NB = BC // 128            # m-tiles per core (8)
R = 425
IN = 20
GN = 4 * R                # 1700 gate columns
GNP = 1712                # padded gate block width (16-mult for fp8 APs)
HC = 1024                 # packed state rows (8 chunks of 128)
NKC = 8                   # state chunks
H1OFF = 512               # h1 row offset in the packed state
NSTEPS = 10
SW = 64.0                 # weight scale (ACT applies 1/SW)
NW_BLK = 12               # W blocks per step: L0 chunks 0..3, L1 chunks 0..7
THREE_STEPS = (0,)        # steps that run the Wl correction pass
# N chunks of the gate output (PSUM-bank sized)
NCHUNKS = [(0, 512), (512, 512), (1024, 512), (1536, 164)]

TRACE = False
LAST_RESULT = None


def build_bass(n_steps=NSTEPS, finalize=True):
    import concourse.bacc as bacc
    import concourse.mybir as mybir
    import concourse.tile as tile

    f32 = mybir.dt.float32
    f16 = mybir.dt.float16
    fp8 = mybir.dt.float8e4
    Sig = mybir.ActivationFunctionType.Sigmoid
    Tanh = mybir.ActivationFunctionType.Tanh
    DR = mybir.MatmulPerfMode.DoubleRow
    MUL = mybir.AluOpType.mult
    ADD = mybir.AluOpType.add
    SUB = mybir.AluOpType.subtract

    three_steps = [t for t in THREE_STEPS if t < n_steps]

    nc = bacc.Bacc()

    wh_d = nc.declare_dram_parameter("wh", [n_steps, 128, NW_BLK * GNP], fp8, False)
    wl_d = nc.declare_dram_parameter("wl", [len(three_steps), 128, NW_BLK * GNP],
                                     fp8, False)
    ht_i = nc.declare_dram_parameter("hti", [128, NKC * BC], f16, False)
    sh_i = nc.declare_dram_parameter("shi", [128, NKC * BC], fp8, False)
    sl_i = nc.declare_dram_parameter("sli", [128, NKC * BC], fp8, False)
    hb_i = nc.declare_dram_parameter("hbi", [128, NB * 1024], f16, False)
    c0_i = nc.declare_dram_parameter("c0i", [128, NB * R], f16, False)
    c1_i = nc.declare_dram_parameter("c1i", [128, NB * R], f16, False)
    out_d = nc.declare_dram_parameter("out", [BC, n_steps * R], f16, True)
    hd = nc.dram_tensor("hd", [BC, 1024], f16)

    with tile.TileContext(nc) as tc:
        with (
            tc.tile_pool(name="consts", bufs=1) as consts,
            tc.tile_pool(name="wpool", bufs=2) as wpool,
            tc.tile_pool(name="gpsum", bufs=2, space="PSUM") as gpsum,
            tc.tile_pool(name="tmp", bufs=3) as tmp,
        ):
            # persistent state
            hT = consts.tile([128, NKC * BC], f16)    # transposed fp16 state
            sh8 = consts.tile([128, NKC * BC], fp8)   # Hh chunks
            sl8 = consts.tile([128, NKC * BC], fp8)   # Hl chunks
            hbm = consts.tile([128, NB * 1024], f16)  # packed batch-major
            c0 = consts.tile([128, NB * R], f16)
            c1 = consts.tile([128, NB * R], f16)
            wl = consts.tile([128, max(1, len(three_steps)) * NW_BLK * GNP], fp8)

            # init DMAs: state on SP queue (urgent first), weights on Pool
            for c in range(NKC):
                cs = slice(c * BC, (c + 1) * BC)
                nc.sync.dma_start(sh8[:, cs], sh_i[:, cs])
            for c in range(NKC):
                cs = slice(c * BC, (c + 1) * BC)
                nc.sync.dma_start(sl8[:, cs], sl_i[:, cs])
            nc.sync.dma_start(hbm[:], hb_i[:])
            nc.sync.dma_start(c0[:], c0_i[:])
            nc.sync.dma_start(c1[:], c1_i[:])
            nc.sync.dma_start(hT[:], ht_i[:])

            w = wpool.tile([128, NW_BLK * GNP], fp8, tag="w")
            for k in range(NW_BLK):
                ks = slice(k * GNP, (k + 1) * GNP)
                nc.gpsimd.dma_start(w[:, ks], wh_d[0][:, ks])
            for t3 in three_steps:
                lo = t3 * NW_BLK * GNP
                for k in range(0, NW_BLK, 3):
                    ks = slice(lo + k * GNP, lo + min(k + 3, NW_BLK) * GNP)
                    kd = slice(k * GNP, min(k + 3, NW_BLK) * GNP)
                    nc.gpsimd.dma_start(wl[:, ks], wl_d[t3][:, kd])

            # PE warm-up (HAM/p-state ramp) on zeroed scratch
            warm = consts.tile([128, 128], mybir.dt.bfloat16)
            nc.vector.memset(warm[:], 0.0)
            wps = gpsum.tile([128, 512], f32, tag="g")
            for i in range(20):
                nc.tensor.matmul(wps[:, 0:128], warm[:], warm[:],
                                 start=True, stop=True)

            def dr_pair(stat, ca, m):
                """Stationary AP [128, 2, 128]: chunks (ca, ca+1), m-tile m."""
                return (stat[:, ca * BC: (ca + 2) * BC]
                        .rearrange("p (two c) -> p two c", two=2)
                        [:, :, m * 128: (m + 1) * 128])

            def w_pair(wt, base, blk, no, nw):
                """Moving AP [128, 2, nw]: W blocks (blk, blk+1) cols no..no+nw."""
                lo = base + blk * GNP
                return (wt[:, lo: lo + 2 * GNP]
                        .rearrange("p (two n) -> p two n", two=2)
                        [:, :, no: no + nw])

            for t in range(n_steps):
                three = t in three_steps
                wl_base = three_steps.index(t) * NW_BLK * GNP if three else 0
                if t < n_steps - 1:
                    w_next = wpool.tile([128, NW_BLK * GNP], fp8, tag="w")
                    for c in range(4):
                        lo = c * 3 * GNP
                        hi = min((c + 1) * 3 * GNP, NW_BLK * GNP)
                        nc.gpsimd.dma_start(w_next[:, lo: hi],
                                            wh_d[t + 1][:, lo: hi])

                for layer in range(2):
                    if layer == 0:
                        # (chunk-pair start, W block index)
                        kplan = [(0, 0), (2, 2)]
                        cst = c0
                    else:
                        # h1 pairs first: ready from step t-1; h0' pairs
                        # stream in from this step's L0 pipeline
                        kplan = [(4, 8), (6, 10), (0, 4), (2, 6)]
                        cst = c1
                    npass = 3 if three else 2
                    nk = len(kplan) * npass
                    for m in range(NB):
                        g = gpsum.tile([128, GN], f32, tag="g")
                        ki = 0
                        for (ca, blk) in kplan:
                            for ps in range(npass):
                                if ps == 0:
                                    stat, wt, base = sh8, w, 0
                                elif ps == 1:
                                    stat, wt, base = sl8, w, 0
                                else:
                                    stat, wt, base = sh8, wl, wl_base
                                lhsT = dr_pair(stat, ca, m)
                                for (no, nw) in NCHUNKS:
                                    nc.tensor.matmul(
                                        g[:, no: no + nw],
                                        lhsT,
                                        w_pair(wt, base, blk, no, nw),
                                        start=(ki == 0),
                                        stop=(ki == nk - 1),
                                        perf_mode=DR,
                                    )
                                ki += 1

                        # LSTM cell (gate order i, f, o, g), fp16 math
                        sg = tmp.tile([128, 3 * R], f16, tag="sg")
                        nc.scalar.activation(sg[:], g[:, 0: 3 * R], Sig,
                                             scale=1.0 / SW)
                        tg = tmp.tile([128, R], f16, tag="tg")
                        nc.scalar.activation(tg[:], g[:, 3 * R: 4 * R], Tanh,
                                             scale=1.0 / SW)
                        si = sg[:, 0: R]
                        sf = sg[:, R: 2 * R]
                        so = sg[:, 2 * R: 3 * R]
                        cs = cst[:, m * R: (m + 1) * R]
                        ta = tmp.tile([128, R], f16, tag="ta")
                        nc.vector.tensor_tensor(out=ta[:], in0=si, in1=tg[:],
                                                op=MUL)
                        tb = tmp.tile([128, R], f16, tag="tb")
                        nc.vector.tensor_tensor(out=tb[:], in0=sf, in1=cs,
                                                op=MUL)
                        nc.vector.tensor_tensor(out=cs, in0=ta[:], in1=tb[:],
                                                op=ADD)
                        tw = tmp.tile([128, R], f16, tag="tw")
                        nc.scalar.activation(tw[:], cs, Tanh)
                        hst = hbm[:, m * 1024 + layer * 512:
                                  m * 1024 + layer * 512 + R]
                        nc.vector.tensor_tensor(out=hst, in0=so, in1=tw[:],
                                                op=MUL)

                        last = (t == n_steps - 1)

                        if layer == 1:
                            nc.sync.dma_start(
                                out_d[m * 128: (m + 1) * 128,
                                      t * R: (t + 1) * R], hst)
                        if not (last and layer == 1):
                            nc.sync.dma_start(
                                hd[m * 128: (m + 1) * 128,
                                   layer * 512: (layer + 1) * 512],
                                hbm[:, m * 1024 + layer * 512:
                                    m * 1024 + (layer + 1) * 512])
                        if (m in (3, 7)) and not (last and layer == 1):
                            lo = {3: 0, 7: 512}[m]
                            hi = {3: 512, 7: 1024}[m]
                            cbase = 0 if layer == 0 else 4
                            for cb in range(4):
                                chunk = cbase + cb
                                dst = slice(chunk * BC + lo, chunk * BC + hi)
                                nc.sync.dma_start(
                                    out=hT[:, dst],
                                    in_=hd[lo: hi,
                                           layer * 512 + cb * 128:
                                           layer * 512 + (cb + 1) * 128],
                                    transpose=True)
                                nc.vector.tensor_copy(sh8[:, dst], hT[:, dst])
                                if cb % 2 == 0:
                                    nc.vector.tensor_tensor(
                                        out=sl8[:, dst], in0=hT[:, dst],
                                        in1=sh8[:, dst], op=SUB)
                                else:
                                    nc.gpsimd.tensor_tensor(
                                        out=sl8[:, dst], in0=hT[:, dst],
                                        in1=sh8[:, dst], op=SUB)
                if t < n_steps - 1:
                    w = w_next
    if finalize:
        nc.finalize()
    return nc


def _pack_pf(a):
    """[BC, C] -> [128, NB*C] with m-tile m at cols m*C."""
    c = a.shape[1]
    return np.ascontiguousarray(
        a.reshape(NB, 128, c).transpose(1, 0, 2).reshape(128, NB * c))


def _pack_kt(a):
    """[HC, BC] (rows=K) -> [128, NKC*BC] with chunk k at cols k*BC."""
    return np.ascontiguousarray(
        a.reshape(NKC, 128, BC).transpose(1, 0, 2).reshape(128, NKC * BC))


def prep_inputs(x, init_states_input, W_i2h0, b_i2h0, W_h2h0, b_h2h0,
                W_i2h1, b_i2h1, W_h2h1, b_h2h1, n_steps=NSTEPS):
    """Host-side packing.  Returns (in_maps, h1_init_full)."""
    x = np.asarray(x, np.float32)
    init = np.asarray(init_states_input, np.float32)
    three_steps = [t for t in THREE_STEPS if t < n_steps]

    # --- weights: W_eff rows match the packed state layout, x SW ---
    Wh_all = np.zeros((n_steps, NW_BLK * 128, GNP), F8)
    Wl_all = np.zeros((max(1, len(three_steps)), NW_BLK * 128, GNP), F8)
    for t in range(n_steps):
        w0 = np.zeros((4 * 128, GN), np.float32)
        w0[0:R] = np.asarray(W_h2h0[t], np.float32).T
        w0[R] = np.asarray(b_i2h0[t], np.float32) + np.asarray(b_h2h0[t], np.float32)
        w0[R + 1: R + 1 + IN] = np.asarray(W_i2h0[t], np.float32).T
        w1 = np.zeros((8 * 128, GN), np.float32)
        w1[0:R] = np.asarray(W_i2h1[t], np.float32).T
        w1[R] = np.asarray(b_i2h1[t], np.float32) + np.asarray(b_h2h1[t], np.float32)
        w1[H1OFF: H1OFF + R] = np.asarray(W_h2h1[t], np.float32).T
        wcat = np.concatenate([w0, w1], axis=0) * SW
        wh = wcat.astype(F8)
        Wh_all[t, :, 0:GN] = wh
        if t in three_steps:
            Wl_all[three_steps.index(t), :, 0:GN] = (
                wcat - wh.astype(np.float32)).astype(F8)

    def wdev(Wblocks):
        n = Wblocks.shape[0]
        return np.ascontiguousarray(
            Wblocks.reshape(n, NW_BLK, 128, GNP).transpose(0, 2, 1, 3)
            .reshape(n, 128, NW_BLK * GNP))

    wh_dev = wdev(Wh_all)
    wl_dev = wdev(Wl_all)

    # --- states ---
    init4 = init.reshape(B, 4, R)
    h0_full, c0_full = init4[:, 0], init4[:, 1]
    h1_full, c1_full = init4[:, 2], init4[:, 3]

    in_maps = []
    for c in range(NCORES):
        sl = slice(c * BC, (c + 1) * BC)
        hcp = np.zeros((BC, 1024), np.float32)
        hcp[:, 0:R] = h0_full[sl]
        hcp[:, R] = 1.0
        hcp[:, R + 1: R + 1 + IN] = x[sl]
        hcp[:, H1OFF: H1OFF + R] = h1_full[sl]
        hcatT = np.zeros((HC, BC), np.float32)
        hcatT[0:R] = h0_full[sl].T
        hcatT[R] = 1.0
        hcatT[R + 1: R + 1 + IN] = x[sl].T
        hcatT[H1OFF: H1OFF + R] = h1_full[sl].T
        ht16 = hcatT.astype(F16)
        sh8 = ht16.astype(F8)
        sl8 = (ht16.astype(np.float32) - sh8.astype(np.float32)).astype(F8)
        in_maps.append({
            "wh": wh_dev,
            "wl": wl_dev,
            "hbi": _pack_pf(hcp.astype(F16)),
            "hti": _pack_kt(ht16),
            "shi": _pack_kt(sh8),
            "sli": _pack_kt(sl8),
            "c0i": _pack_pf(np.ascontiguousarray(c0_full[sl]).astype(F16)),
            "c1i": _pack_pf(np.ascontiguousarray(c1_full[sl]).astype(F16)),
        })
    return in_maps, h1_full


def kernel(x, init_states_input, W_i2h0, b_i2h0, W_h2h0, b_h2h0,
           W_i2h1, b_i2h1, W_h2h1, b_h2h1):
    global LAST_RESULT
    from concourse.bass_utils import run_bass_kernel_spmd

    in_maps, h1_full = prep_inputs(
        x, init_states_input, W_i2h0, b_i2h0, W_h2h0, b_h2h0,
        W_i2h1, b_i2h1, W_h2h1, b_h2h1)

    nc = build_bass(NSTEPS)
    res = run_bass_kernel_spmd(nc, in_maps, list(range(NCORES)), trace=TRACE)
    LAST_RESULT = res

    out = np.empty((B, (NSTEPS + 1) * R), np.float32)
    out[:, 0:R] = h1_full
    for c in range(NCORES):
        out[c * BC: (c + 1) * BC, R:] = res.results[c]["out"].astype(np.float32)
    return out
